# revision 1
# baseline (speedup 1.0000x reference)
"""Trainium2 Bass kernel for DepthwiseSeparableConv (depthwise 3x3 shared-kernel
conv -> channels-last memory-reinterpret -> pointwise 1x1 conv -> ReLU -> sync
BatchNorm), data-parallel over batch across 8 NeuronCores.

Self-contained: hardcodes shapes from the problem spec; imports only the
system-installed `concourse` (Bass/Tile) stack.

Algorithm per core (8 of 64 batches, processed in 2 groups of 4):
  1. Load x[b] [128c, 3136n] f32 to SBUF (two spatial halves per batch),
     PE-transpose in 112-column blocks to spatial-major xt tiles
     [112n, 4*128c] f32r (4 batches side by side -> 512 moving rows).
  2. Depthwise conv as banded matmuls: out_blk(i) = sum_d A_d^T @ xt_blk(i+d),
     d in {-1,0,+1}; A_d are [112,112] banded matrices built on device from the
     9 shared taps with affine_select (w-border validity exact; h-borders are
     handled by skipping the missing-neighbor matmul at i=0 / i=27).
  3. Conv output (spatial-major Z[b] = [3136, 128]) is rounded to bf16 and
     bounced through DRAM; reading it back as a flat [128, 3136] view realizes
     the reference's memory-reinterpretation y = Z.flat.reshape(128, 3136).
  4. Pointwise: out = pw @ y via PE (bf16 in, f32 accum), ReLU fused into the
     PSUM->SBUF copy on ScalarE with accumulated per-partition sums; squared
     sums via DVE scalar_tensor_tensor accum. Pre-BN activations stay resident
     in SBUF as bf16.
  5. Per-channel (sum, sumsq) all-reduced across the 8 cores (gpsimd
     collective), exact biased-variance BN affine applied on GpSimd from the
     bf16 residents, f32 written out.
"""

import os
import numpy as np
from contextlib import ExitStack

import concourse.bass as bass
import concourse.bacc as bacc
import concourse.mybir as mybir
from concourse import tile
from concourse.bass_utils import run_bass_kernel_spmd

F32 = mybir.dt.float32
F32R = mybir.dt.float32r
BF16 = mybir.dt.bfloat16

B, CIN, COUT, H, W = 64, 128, 256, 56, 56
HW = H * W              # 3136
BLK = 112               # conv block rows (2*W)
NBLK = HW // BLK        # 28
NCORES = 8
BPC = B // NCORES       # 8 batches per core
GB = 4                  # batches per conv group
NGRP = BPC // GB        # 2
HHW = HW // 2           # 1568 (x half-tile columns)
HBLK = NBLK // 2        # 14 blocks per half
NCH = 448               # pointwise moving chunk
NJ = HW // NCH          # 7
EPS = 1e-5
NTOT = float(B * HW)    # BN count

# tap index: s = 56*dh + dw  ->  k9[3*(dh+1) + (dw+1)]
TAPS = [(56 * dh + dw, 3 * (dh + 1) + (dw + 1), dw)
        for dh in (-1, 0, 1) for dw in (-1, 0, 1)]


def _build_A(nc, k_sb, const, tmp_pool):
    """Build A_delta [128(part n_in, 112 used), 112(n_out)] f32r tiles."""
    parts = {d: [] for d in (-1, 0, 1)}
    for d in (-1, 0, 1):
        for s, k9i, dw in TAPS:
            D = s - 112 * d
            if not (-111 <= D <= 111):
                continue
            t = tmp_pool.tile([128, BLK], F32, tag="abuild", bufs=30,
                              name=f"ab{d}_{s}")
            # diagonal: keep k where p - f - D == 0 (f = 56*i1 + i2)
            nc.gpsimd.affine_select(
                out=t[:], in_=k_sb[:, k9i:k9i + 1].broadcast_to((128, BLK)),
                pattern=[[-56, 2], [-1, 56]], base=-D,
                compare_op=mybir.AluOpType.is_equal, fill=0.0,
                channel_multiplier=1)
            if dw == 1:    # need (f % 56) <= 54: keep where 54 - i2 >= 0
                nc.gpsimd.affine_select(
                    out=t[:], in_=t[:], pattern=[[0, 2], [-1, 56]], base=54,
                    compare_op=mybir.AluOpType.is_ge, fill=0.0,
                    channel_multiplier=0)
            elif dw == -1:  # need (f % 56) >= 1: keep where i2 > 0
                nc.gpsimd.affine_select(
                    out=t[:], in_=t[:], pattern=[[0, 2], [1, 56]], base=0,
                    compare_op=mybir.AluOpType.is_gt, fill=0.0,
                    channel_multiplier=0)
            parts[d].append(t)
    A = {}
    for d in (-1, 0, 1):
        af = tmp_pool.tile([128, BLK], F32, tag="abuildacc", bufs=3,
                           name=f"af{d}")
        ts = parts[d]
        nc.gpsimd.tensor_tensor(af[:], ts[0][:], ts[1][:], mybir.AluOpType.add)
        for t in ts[2:]:
            nc.gpsimd.tensor_tensor(af[:], af[:], t[:], mybir.AluOpType.add)
        ar = const.tile([128, BLK], BF16, tag=f"A{d}", name=f"Ar{d}")
        nc.gpsimd.tensor_copy(ar[:], af[:])
        A[d] = ar
    return A


def build_nc():
    nc = bacc.Bacc(num_devices=NCORES)

    x_in = nc.declare_dram_parameter("x", [BPC, CIN, HW], BF16, isOutput=False)
    dwk = nc.declare_dram_parameter("dwk", [9], F32, isOutput=False)
    pwt = nc.declare_dram_parameter("pwT", [CIN, COUT], F32, isOutput=False)
    gam = nc.declare_dram_parameter("gamma", [COUT], F32, isOutput=False)
    bet = nc.declare_dram_parameter("beta", [COUT], F32, isOutput=False)
    out = nc.declare_dram_parameter("out", [BPC, COUT, HW], F32, isOutput=True)

    with ExitStack() as ctx:
        tc = ctx.enter_context(tile.TileContext(nc))
        const = ctx.enter_context(tc.tile_pool(name="const", bufs=1))
        xtpool = ctx.enter_context(tc.tile_pool(name="xt", bufs=5))
        # (pools below opened after the A-build scratch pool is released)
        zpool = ctx.enter_context(tc.tile_pool(name="z", bufs=4))
        ypool = ctx.enter_context(tc.tile_pool(name="y", bufs=2))
        opool = ctx.enter_context(tc.tile_pool(name="o", bufs=2))
        respool = ctx.enter_context(tc.tile_pool(name="res", bufs=2 * BPC))
        scpool = ctx.enter_context(tc.tile_pool(name="scr", bufs=2))
        ps_t = ctx.enter_context(tc.tile_pool(name="ps_t", bufs=2, space="PSUM"))
        ps_c = ctx.enter_context(tc.tile_pool(name="ps_c", bufs=2, space="PSUM"))
        ps_p = ctx.enter_context(tc.tile_pool(name="ps_p", bufs=2, space="PSUM"))
        dram = ctx.enter_context(tc.tile_pool(name="dram", bufs=1, space="DRAM"))

        stage = int(os.environ.get("BASS_STAGE", "3"))
        no_cc = bool(os.environ.get("BASS_NO_CC"))

        # ---- constants (A-build scratch pool closed before the big pools) ----
        k_sb = const.tile([128, 9], F32, tag="k")
        nc.sync.dma_start(k_sb[:], dwk.ap().partition_broadcast(128))

        ident = const.tile([128, 128], BF16, tag="ident")
        pw_f32 = const.tile([128, COUT], F32, tag="pwf")
        nc.sync.dma_start(pw_f32[:], pwt[:, :])
        pw_sb = const.tile([128, COUT], BF16, tag="pw")
        nc.vector.tensor_copy(pw_sb[:], pw_f32[:])

        gb_sb = const.tile([128, 4], F32, tag="gb")
        nc.sync.dma_start(gb_sb[:, 0:2], gam.ap().rearrange("(j p) -> p j", p=128))
        nc.sync.dma_start(gb_sb[:, 2:4], bet.ap().rearrange("(j p) -> p j", p=128))

        ones = const.tile([128, 1], F32, tag="ones")
        nc.gpsimd.memset(ones[:], 1.0)
        with tc.tile_pool(name="abuild", bufs=2) as tmp_pool:
            identf = tmp_pool.tile([128, 128], F32, tag="identf", bufs=1)
            nc.gpsimd.affine_select(
                out=identf[:], in_=ones[:].broadcast_to((128, 128)),
                pattern=[[-1, 128]], base=0,
                compare_op=mybir.AluOpType.is_equal, fill=0.0,
                channel_multiplier=1)
            nc.gpsimd.tensor_copy(ident[:], identf[:])
            A = _build_A(nc, k_sb, const, tmp_pool)

        xpool = ctx.enter_context(tc.tile_pool(name="x", bufs=8))

        # stats slots: per oc, one column per (batch, chunk-pair)
        NSL = 4
        sums = [const.tile([128, BPC * NSL], F32, tag=f"sum{oc}",
                           name=f"sums{oc}") for oc in range(2)]
        sqs = [const.tile([128, BPC * NSL], F32, tag=f"sq{oc}",
                          name=f"sqs{oc}") for oc in range(2)]

        zscr = [dram.tile([GB, HW, CIN], BF16, tag=f"zg{g}", name=f"zscr{g}")
                for g in range(NGRP)]
        st_in = dram.tile([128, 4], F32, tag="stin")
        st_out = dram.tile([128, 4], F32, tag="stout")

        res_tiles = [[None] * 2 for _ in range(BPC)]

        # ---- phase 1 ----
        for g in range(NGRP):
            # x half-tiles: xh[h][b4] covers columns [h*1568, (h+1)*1568)
            xh = [[None] * GB for _ in range(2)]
            for h in range(2):
                for b4 in range(GB):
                    xt_ = xpool.tile([128, HHW], BF16, tag="x",
                                     name=f"xh{g}_{h}_{b4}")
                    nc.sync.dma_start(
                        xt_[:], x_in[GB * g + b4, :, HHW * h:HHW * (h + 1)])
                    xh[h][b4] = xt_

            xt_tiles = {}

            def conv_block(i):
                ps = ps_c.tile([BLK, GB * 128], F32, tag="cv")
                deltas = [d for d in (-1, 0, 1) if 0 <= i + d < NBLK]
                for idx, d in enumerate(deltas):
                    nc.tensor.matmul(
                        ps[:], A[d][0:BLK, :], xt_tiles[i + d][:],
                        start=(idx == 0), stop=(idx == len(deltas) - 1))
                z_sb = zpool.tile([BLK, GB * 128], BF16, tag="z")
                nc.scalar.activation(z_sb[:], ps[:],
                                     mybir.ActivationFunctionType.Copy)
                nc.sync.dma_start(
                    zscr[g][0:GB, BLK * i:BLK * (i + 1), :]
                    .rearrange("b r c -> r b c"),
                    z_sb[:])

            for i in range(NBLK):
                h, li = divmod(i, HBLK)
                tps = ps_t.tile([BLK, GB * 128], BF16, tag="tp")
                for b4 in range(GB):
                    nc.tensor.transpose(
                        tps[:, 128 * b4:128 * (b4 + 1)],
                        xh[h][b4][:, BLK * li:BLK * (li + 1)],
                        ident[:])
                xt_sb = xtpool.tile([BLK, GB * 128], BF16, tag="xt")
                nc.vector.tensor_copy(xt_sb[:], tps[:])
                xt_tiles[i] = xt_sb
                if i >= 1:
                    conv_block(i - 1)
            conv_block(NBLK - 1)

            # pointwise for this group's batches
            for b4 in range(GB):
                b = GB * g + b4
                if stage == 1:
                    continue
                y_sb = ypool.tile([128, HW], BF16, tag="y")
                nc.sync.dma_start(
                    y_sb[:],
                    zscr[g][b4].flatten().rearrange("(p n) -> p n", p=128))
                for oc in range(2):
                    res = respool.tile([128, HW], BF16, tag="res")
                    res_tiles[b][oc] = res
                    # chunk pairs: (0,1) (2,3) (4,5) (6,) -> 4 relu/sq ops.
                    # each matmul lands in its own 512-f32 PSUM bank slot.
                    for jj, js in enumerate([(0, 1), (2, 3), (4, 5), (6,)]):
                        w = NCH * len(js)
                        ps = ps_p.tile([128, 1024], F32, tag="pw")
                        for k, j in enumerate(js):
                            nc.tensor.matmul(
                                ps[:, 512 * k:512 * k + NCH],
                                pw_sb[:, 128 * oc:128 * (oc + 1)],
                                y_sb[:, NCH * j:NCH * (j + 1)],
                                start=True, stop=True)
                        slot = b * NSL + jj
                        j0 = js[0]
                        if len(js) == 2:
                            ps_in = (ps[:].rearrange("p (k c) -> p k c", c=512)
                                     [:, :, 0:NCH])
                            rs = (res[:, NCH * j0:NCH * j0 + w]
                                  .rearrange("p (k c) -> p k c", c=NCH))
                        else:
                            ps_in = ps[:, 0:NCH]
                            rs = res[:, NCH * j0:NCH * j0 + w]
                        sc = scpool.tile([128, 2 * NCH], BF16, tag="scr")
                        scv = (sc[:, 0:w].rearrange("p (k c) -> p k c", c=NCH)
                               if len(js) == 2 else sc[:, 0:w])
                        if (b + oc) % 2 == 0:
                            nc.scalar.activation(
                                rs, ps_in,
                                mybir.ActivationFunctionType.Relu,
                                accum_out=sums[oc][:, slot:slot + 1])
                            nc.vector.scalar_tensor_tensor(
                                out=scv, in0=rs, scalar=1.0, in1=rs,
                                op0=mybir.AluOpType.mult,
                                op1=mybir.AluOpType.mult,
                                accum_out=sqs[oc][:, slot:slot + 1])
                        else:
                            nc.vector.tensor_scalar(
                                rs, ps_in, 0.0, 0.0,
                                mybir.AluOpType.max, mybir.AluOpType.add,
                                accum_out=sums[oc][:, slot:slot + 1])
                            nc.scalar.activation(
                                scv, rs,
                                mybir.ActivationFunctionType.Square,
                                accum_out=sqs[oc][:, slot:slot + 1])

        if stage == 1:
            for b in range(BPC):
                zview = (zscr[b // GB][b % GB].flatten()
                         .rearrange("(p n) -> p n", p=128))
                o_sb = opool.tile([128, HW], F32, tag="o", name=f"od{b}")
                y_dbg = ypool.tile([128, HW], BF16, tag="y", name=f"yd{b}")
                nc.sync.dma_start(y_dbg[:], zview)
                nc.vector.tensor_copy(o_sb[:], y_dbg[:])
                nc.sync.dma_start(out[b, 0:128, :], o_sb[:])
                nc.sync.dma_start(out[b, 128:256, :], o_sb[:])
        if stage == 2:
            for b in range(BPC):
                for oc in range(2):
                    o_sb = opool.tile([128, HW], F32, tag="o",
                                      name=f"os{b}_{oc}")
                    nc.vector.tensor_copy(o_sb[:], res_tiles[b][oc][:])
                    nc.sync.dma_start(out[b, 128 * oc:128 * (oc + 1), :],
                                      o_sb[:])

        # ---- stats: local reduce -> all-reduce -> affine params ----
        red = const.tile([128, 4], F32, tag="red")
        allr = const.tile([128, 4], F32, tag="allr")
        me = const.tile([128, 4], F32, tag="me")    # mean0 mean1 msq0 msq1
        var = const.tile([128, 2], F32, tag="var")
        std = const.tile([128, 2], F32, tag="std")
        rstd = const.tile([128, 2], F32, tag="rstd")
        sc_b = const.tile([128, 4], F32, tag="scb")  # scale0 scale1 nbias0 nbias1
        if stage >= 3:
            for oc in range(2):
                nc.vector.tensor_reduce(red[:, oc:oc + 1], sums[oc][:],
                                        axis=mybir.AxisListType.X,
                                        op=mybir.AluOpType.add)
                nc.vector.tensor_reduce(red[:, 2 + oc:3 + oc], sqs[oc][:],
                                        axis=mybir.AxisListType.X,
                                        op=mybir.AluOpType.add)
            nc.sync.dma_start(st_in[:], red[:])
            if no_cc:
                nc.sync.dma_start(st_out[:], st_in[:])
            else:
                nc.gpsimd.collective_compute(
                    "AllReduce", mybir.AluOpType.add,
                    replica_groups=[list(range(NCORES))],
                    ins=[st_in[:].opt()], outs=[st_out[:].opt()])
            nc.sync.dma_start(allr[:], st_out[:])

            nc.vector.tensor_scalar(me[:], allr[:],
                                    (8.0 if no_cc else 1.0) / NTOT, None,
                                    mybir.AluOpType.mult)
            nc.vector.tensor_tensor(var[:], me[:, 0:2], me[:, 0:2],
                                    mybir.AluOpType.mult)
            nc.vector.tensor_tensor(var[:], me[:, 2:4], var[:],
                                    mybir.AluOpType.subtract)
            nc.vector.tensor_scalar(var[:], var[:], EPS, None,
                                    mybir.AluOpType.add)
            nc.scalar.activation(std[:], var[:],
                                 mybir.ActivationFunctionType.Sqrt)
            nc.vector.reciprocal(rstd[:], std[:])
            nc.vector.tensor_tensor(sc_b[:, 0:2], rstd[:], gb_sb[:, 0:2],
                                    mybir.AluOpType.mult)
            nc.vector.tensor_tensor(sc_b[:, 2:4], me[:, 0:2], sc_b[:, 0:2],
                                    mybir.AluOpType.mult)
            nc.vector.tensor_tensor(sc_b[:, 2:4], gb_sb[:, 2:4], sc_b[:, 2:4],
                                    mybir.AluOpType.subtract)

        # ---- phase 2: affine + writeout (split Pool/DVE/ACT) ----
        for b in range(BPC) if stage >= 3 else []:
            for oc in range(2):
                o_sb = opool.tile([128, HW], F32, tag="o")
                sel = (2 * b + oc) % 4
                if sel == 3:
                    nc.scalar.activation(
                        o_sb[:], res_tiles[b][oc][:],
                        mybir.ActivationFunctionType.Identity,
                        bias=sc_b[:, 2 + oc:3 + oc],
                        scale=sc_b[:, oc:oc + 1])
                elif sel == 2:
                    nc.vector.tensor_scalar(
                        o_sb[:], res_tiles[b][oc][:],
                        sc_b[:, oc:oc + 1], sc_b[:, 2 + oc:3 + oc],
                        mybir.AluOpType.mult, mybir.AluOpType.add)
                else:
                    nc.gpsimd.tensor_scalar(
                        o_sb[:], res_tiles[b][oc][:],
                        sc_b[:, oc:oc + 1], sc_b[:, 2 + oc:3 + oc],
                        mybir.AluOpType.mult, mybir.AluOpType.add)
                nc.sync.dma_start(out[b, 128 * oc:128 * (oc + 1), :], o_sb[:])

    nc.finalize()
    return nc


_NC_CACHE = []


def kernel(x, dw_w, pw_w, gamma, beta):
    import ml_dtypes
    x = np.ascontiguousarray(
        np.asarray(x, dtype=np.float32).astype(ml_dtypes.bfloat16)
    ).reshape(B, CIN, HW)
    dwk = np.ascontiguousarray(np.asarray(dw_w, dtype=np.float32)).reshape(9)
    pwT = np.ascontiguousarray(np.asarray(pw_w, dtype=np.float32).T)
    gamma = np.ascontiguousarray(np.asarray(gamma, dtype=np.float32))
    beta = np.ascontiguousarray(np.asarray(beta, dtype=np.float32))

    if not _NC_CACHE:
        _NC_CACHE.append(build_nc())
    nc = _NC_CACHE[0]

    in_maps = []
    for r in range(NCORES):
        shard = np.ascontiguousarray(x[r * BPC:(r + 1) * BPC])
        in_maps.append({"x": shard, "dwk": dwk, "pwT": pwT,
                        "gamma": gamma, "beta": beta})

    br = run_bass_kernel_spmd(nc, in_maps, list(range(NCORES)))
    outs = [br.results[r]["out"].reshape(BPC, COUT, H, W) for r in range(NCORES)]
    return np.concatenate(outs, axis=0)



# revision 5
# speedup vs baseline: 1.1804x; 1.1804x over previous
"""Trainium2 Bass kernel for DepthwiseSeparableConv (depthwise 3x3 shared-kernel
conv -> channels-last memory-reinterpret -> pointwise 1x1 conv -> ReLU -> sync
BatchNorm), data-parallel over batch across 8 NeuronCores.

Self-contained: hardcodes shapes from the problem spec; imports only the
system-installed `concourse` (Bass/Tile) stack.

Key layout trick vs the naive scheme: conv blocks are STRIDED, block l holds
spatial rows n = 28q + l (q = 0..111 partitions, l = 0..27 blocks). The 3x3
stencil still reduces to 3 banded matmuls per block (bands at q-offsets
{-2,0,+2} <-> dh, block delta <-> dw); h-borders drop out automatically via
band clipping, w-borders only touch the two wrap matrices (parity masks on q).
Because q is the SLOW index of the channels-last flat order
(F = 3584 q + 128 l + c), the conv output can be written to DRAM with 3584B
contiguous runs (full DMA speed) and read back as y = [128, 3136] contiguous,
realizing the reference's memory reinterpretation with ~1/3 the DMA cost of a
256B-chunked bounce.

Per core (8 of 64 batches, 2 groups of 4):
  1. Load x[b] [128c, 3136n] bf16. PE-transpose strided column sets
     x[:, l::28] -> xt_l [112q, 4b*128c] bf16 (DVE copies PSUM->SBUF).
  2. Conv: ps_l = A_{-1}^T xt_{l-1} + A_0^T xt_l + A_{+1}^T xt_{l+1} (wrap
     matrices for l=0/27), ACT copies f32 PSUM -> bf16 zg half-tiles.
  3. Bounce zg -> DRAM (4 half-writes/group, 3584B runs) and read back
     y[b] = [128, 3136] bf16 (contiguous).
  4. Pointwise out = pw @ y on PE (bf16 in, f32 accum); ReLU fused into
     PSUM->SBUF with per-channel sum accumulators; squares via second pass
     with sumsq accumulators. Pre-BN activations stay resident in SBUF bf16.
  5. Per-channel (sum, sumsq) AllGather across 8 cores + local reduce (exact
     sync-BN), affine applied on DVE/ACT/Pool, bf16 written out (host widens
     to f32).
"""

import os
import numpy as np
from contextlib import ExitStack

import concourse.bass as bass
import concourse.bacc as bacc
import concourse.mybir as mybir
from concourse import tile
from concourse.bass_utils import run_bass_kernel_spmd

F32 = mybir.dt.float32
BF16 = mybir.dt.bfloat16

B, CIN, COUT, H, W = 64, 128, 256, 56, 56
HW = H * W              # 3136
Q = 112                 # rows per strided block (n = 28q + l)
NBLK = 28               # blocks per image
HBLK = NBLK // 2        # 14 blocks per z half-tile
NCORES = 8
BPC = B // NCORES       # 8 batches per core
GB = 4                  # batches per conv group
NGRP = BPC // GB        # 2
NCH = 448               # pointwise moving chunk
EPS = 1e-5
NTOT = float(B * HW)    # BN count
NSL = 4                 # stats slots per (batch, oc)


def _build_A(nc, k_sb, const, tmp_pool):
    """Build the 5 banded conv matrices [128part(q_in, 112 used), 112(q_out)]
    bf16: A[dw] plain (bands q_in-q_out = 2*dh, coeff k[3*(dh+1)+(dw+1)]),
    plus wrap variants A[-1]w (shifts 2dh-1, odd q_out only; used by block 0
    reading xt_27) and A[+1]w (shifts 2dh+1, even q_out only; block 27
    reading xt_0)."""
    specs = {}
    for dw in (-1, 0, 1):
        specs[f"A{dw}"] = [(2 * dh, 3 * (dh + 1) + (dw + 1), None)
                           for dh in (-1, 0, 1)]
    specs["Am1w"] = [(2 * dh - 1, 3 * (dh + 1) + 0, 1) for dh in (-1, 0, 1)]
    specs["Ap1w"] = [(2 * dh + 1, 3 * (dh + 1) + 2, 0) for dh in (-1, 0, 1)]

    out = {}
    for nm, taps in specs.items():
        parts = []
        for shift, k9i, parity in taps:
            t = tmp_pool.tile([128, Q], F32, tag="abuild", bufs=16,
                              name=f"ab_{nm}_{shift}")
            # diagonal: keep k where p - f - shift == 0 (f = 2*i1 + i2)
            nc.gpsimd.affine_select(
                out=t[:], in_=k_sb[:, k9i:k9i + 1].broadcast_to((128, Q)),
                pattern=[[-2, 56], [-1, 2]], base=-shift,
                compare_op=mybir.AluOpType.is_equal, fill=0.0,
                channel_multiplier=1)
            if parity is not None:
                # keep only columns with f % 2 == parity (f = 56*? no:
                # decompose f = 2*i1 + i2 -> value = i2)
                nc.gpsimd.affine_select(
                    out=t[:], in_=t[:], pattern=[[0, 56], [1, 2]],
                    base=-parity, compare_op=mybir.AluOpType.is_equal,
                    fill=0.0, channel_multiplier=0)
            parts.append(t)
        af = tmp_pool.tile([128, Q], F32, tag="abuildacc", bufs=2,
                           name=f"af_{nm}")
        nc.gpsimd.tensor_tensor(af[:], parts[0][:], parts[1][:],
                                mybir.AluOpType.add)
        nc.gpsimd.tensor_tensor(af[:], af[:], parts[2][:],
                                mybir.AluOpType.add)
        ar = const.tile([128, Q], BF16, tag=f"Ar_{nm}", name=f"Ar_{nm}")
        nc.gpsimd.tensor_copy(ar[:], af[:])
        out[nm] = ar
    return out


def build_nc():
    nc = bacc.Bacc(num_devices=NCORES)

    x_in = nc.declare_dram_parameter("x", [BPC, CIN, HW], BF16, isOutput=False)
    dwk = nc.declare_dram_parameter("dwk", [9], F32, isOutput=False)
    pwt = nc.declare_dram_parameter("pwT", [CIN, COUT], F32, isOutput=False)
    gam = nc.declare_dram_parameter("gamma", [COUT], F32, isOutput=False)
    bet = nc.declare_dram_parameter("beta", [COUT], F32, isOutput=False)
    out = nc.declare_dram_parameter("out", [BPC, COUT, HW], BF16, isOutput=True)

    with ExitStack() as ctx:
        tc = ctx.enter_context(tile.TileContext(nc))
        const = ctx.enter_context(tc.tile_pool(name="const", bufs=1))
        xtpool = ctx.enter_context(tc.tile_pool(name="xt", bufs=4))
        zgpool = ctx.enter_context(tc.tile_pool(name="zg", bufs=2))
        ypool = ctx.enter_context(tc.tile_pool(name="y", bufs=3))
        respool = ctx.enter_context(tc.tile_pool(name="res", bufs=2 * BPC))
        scpool = ctx.enter_context(tc.tile_pool(name="scr", bufs=2))
        ps_t = ctx.enter_context(tc.tile_pool(name="ps_t", bufs=2, space="PSUM"))
        ps_c = ctx.enter_context(tc.tile_pool(name="ps_c", bufs=2, space="PSUM"))
        ps_p = ctx.enter_context(tc.tile_pool(name="ps_p", bufs=2, space="PSUM"))
        dram = ctx.enter_context(tc.tile_pool(name="dram", bufs=1, space="DRAM"))

        no_cc = bool(os.environ.get("BASS_NO_CC"))

        # ---- constants ----
        k_sb = const.tile([128, 9], F32, tag="k")
        nc.sync.dma_start(k_sb[:], dwk.ap().partition_broadcast(128))

        ident = const.tile([128, 128], BF16, tag="ident")
        pw_f32 = const.tile([128, COUT], F32, tag="pwf")
        nc.sync.dma_start(pw_f32[:], pwt[:, :])
        pw_sb = const.tile([128, COUT], BF16, tag="pw")
        nc.vector.tensor_copy(pw_sb[:], pw_f32[:])

        gb_sb = const.tile([128, 4], F32, tag="gb")
        nc.sync.dma_start(gb_sb[:, 0:2], gam.ap().rearrange("(j p) -> p j", p=128))
        nc.sync.dma_start(gb_sb[:, 2:4], bet.ap().rearrange("(j p) -> p j", p=128))

        ones = const.tile([128, 1], F32, tag="ones")
        nc.gpsimd.memset(ones[:], 1.0)
        with tc.tile_pool(name="abuild", bufs=2) as tmp_pool:
            identf = tmp_pool.tile([128, 128], F32, tag="identf", bufs=1)
            nc.gpsimd.affine_select(
                out=identf[:], in_=ones[:].broadcast_to((128, 128)),
                pattern=[[-1, 128]], base=0,
                compare_op=mybir.AluOpType.is_equal, fill=0.0,
                channel_multiplier=1)
            nc.gpsimd.tensor_copy(ident[:], identf[:])
            A = _build_A(nc, k_sb, const, tmp_pool)

        xpool = ctx.enter_context(tc.tile_pool(name="x", bufs=5))

        # stats slots: per oc, one column per (batch, chunk-group)
        sums = [const.tile([128, BPC * NSL], F32, tag=f"sum{oc}",
                           name=f"sums{oc}") for oc in range(2)]
        sqs = [const.tile([128, BPC * NSL], F32, tag=f"sq{oc}",
                          name=f"sqs{oc}") for oc in range(2)]

        # DRAM bounce scratch: zscr[g][b4] flat == y[b] flat (n-major, 128c)
        zscr = [dram.tile([GB, Q, NBLK * 128], BF16, tag=f"zg{g}",
                          name=f"zscr{g}") for g in range(NGRP)]
        st_in = dram.tile([128, 4], F32, tag="stin")
        st_gather = dram.tile([NCORES, 128, 4], F32, tag="stg")

        res_tiles = [[None] * 2 for _ in range(BPC)]
        x_tiles = [None] * BPC
        y_tiles = [None] * BPC

        def load_x(b):
            xt_ = xpool.tile([128, HW], BF16, tag="x", name=f"xh{b}")
            nc.sync.dma_start(xt_[:], x_in[b, :, :])
            x_tiles[b] = xt_

        for b in range(GB + 1):
            load_x(b)

        # ---------- conv group emission (generator: yields per block) ----------
        def emit_conv(g):
            xt_tiles = {}
            zgh = [None, None]
            b0 = GB * g

            def transpose_block(l):
                tps = ps_t.tile([Q, GB * 128], BF16, tag="tp")
                for b4 in range(GB):
                    xv = (x_tiles[b0 + b4][:, :]
                          .rearrange("c (q l) -> l c q", q=Q, l=NBLK)[l])
                    nc.tensor.transpose(
                        tps[:, 128 * b4:128 * (b4 + 1)], xv, ident[:])
                xt_sb = xtpool.tile([Q, GB * 128], BF16,
                                    tag=("xt27" if l == 27 else
                                         "xt0" if l == 0 else "xt"),
                                    bufs=(1 if l in (0, 27) else 4),
                                    name=f"xt{g}_{l}")
                nc.vector.tensor_copy(xt_sb[:], tps[:])
                xt_tiles[l] = xt_sb

            def conv_block(l):
                h = l // HBLK
                if zgh[h] is None:
                    zgh[h] = zgpool.tile([Q, GB * HBLK * 128], BF16, tag="zg",
                                         name=f"zgt{g}_{h}")
                ps = ps_c.tile([Q, GB * 128], F32, tag="cv")
                if l == 0:
                    mats = [(A["Am1w"], 27), (A["A0"], 0), (A["A1"], 1)]
                elif l == NBLK - 1:
                    mats = [(A["A-1"], l - 1), (A["A0"], l), (A["Ap1w"], 0)]
                else:
                    mats = [(A["A-1"], l - 1), (A["A0"], l), (A["A1"], l + 1)]
                for idx, (a, src) in enumerate(mats):
                    nc.tensor.matmul(
                        ps[:], a[0:Q, :], xt_tiles[src][:],
                        start=(idx == 0), stop=(idx == 2))
                zv = (zgh[h][:, :]
                      .rearrange("q (b l c) -> l q b c", b=GB, l=HBLK, c=128)
                      [l % HBLK])
                nc.scalar.activation(zv, ps[:],
                                     mybir.ActivationFunctionType.Copy)
                if l % HBLK == HBLK - 1:
                    nc.sync.dma_start(
                        zscr[g][:, :, 1792 * h:1792 * (h + 1)]
                        .rearrange("b q s -> q b s"),
                        zgh[h][:, :].rearrange("q (b lc) -> q b lc", b=GB))
                    zgh[h] = None

            transpose_block(27)
            yield
            transpose_block(0)
            yield
            for l in range(1, NBLK):
                transpose_block(l)
                conv_block(l - 1)
                yield
            conv_block(NBLK - 1)
            yield

        # ---------- pointwise emission (per (b, oc) unit) ----------
        def emit_pointwise(g):
            for b4 in range(GB):
                b = GB * g + b4
                y_sb = ypool.tile([128, HW], BF16, tag="y", name=f"y{b}")
                nc.sync.dma_start(
                    y_sb[:],
                    zscr[g][b4].flatten().rearrange("(p n) -> p n", p=128))
                y_tiles[b] = y_sb
                for oc in range(2):
                    res = respool.tile([128, HW], BF16, tag="res")
                    res_tiles[b][oc] = res
                    for jj, js in enumerate([(0, 1), (2, 3), (4, 5), (6,)]):
                        w = NCH * len(js)
                        ps = ps_p.tile([128, 1024], F32, tag="pw")
                        for k, j in enumerate(js):
                            nc.tensor.matmul(
                                ps[:, 512 * k:512 * k + NCH],
                                pw_sb[:, 128 * oc:128 * (oc + 1)],
                                y_sb[:, NCH * j:NCH * (j + 1)],
                                start=True, stop=True)
                        slot = b * NSL + jj
                        j0 = js[0]
                        if len(js) == 2:
                            ps_in = (ps[:].rearrange("p (k c) -> p k c", c=512)
                                     [:, :, 0:NCH])
                            rs = (res[:, NCH * j0:NCH * j0 + w]
                                  .rearrange("p (k c) -> p k c", c=NCH))
                        else:
                            ps_in = ps[:, 0:NCH]
                            rs = res[:, NCH * j0:NCH * j0 + w]
                        sc = scpool.tile([128, 2 * NCH], BF16, tag="scr")
                        scv = (sc[:, 0:w].rearrange("p (k c) -> p k c", c=NCH)
                               if len(js) == 2 else sc[:, 0:w])
                        if (b + oc) % 2 == 0:
                            nc.scalar.activation(
                                rs, ps_in,
                                mybir.ActivationFunctionType.Relu,
                                accum_out=sums[oc][:, slot:slot + 1])
                            nc.vector.scalar_tensor_tensor(
                                out=scv, in0=rs, scalar=1.0, in1=rs,
                                op0=mybir.AluOpType.mult,
                                op1=mybir.AluOpType.mult,
                                accum_out=sqs[oc][:, slot:slot + 1])
                        else:
                            nc.vector.tensor_scalar(
                                rs, ps_in, 0.0, 0.0,
                                mybir.AluOpType.max, mybir.AluOpType.add,
                                accum_out=sums[oc][:, slot:slot + 1])
                            nc.scalar.activation(
                                scv, rs,
                                mybir.ActivationFunctionType.Square,
                                accum_out=sqs[oc][:, slot:slot + 1])
                    yield

        # ---------- schedule: g0 conv | g1 conv x g0 pw interleave | g1 pw ----
        for _ in emit_conv(0):
            pass
        for b in range(GB + 1, BPC):
            load_x(b)
        pw0 = emit_pointwise(0)
        step = 0
        for _ in emit_conv(1):
            step += 1
            if step % 2 == 0:
                next(pw0, None)
        for _ in pw0:
            pass
        for _ in emit_pointwise(1):
            pass

        # ---- stats: local reduce -> AllGather -> local sum -> affine params --
        red = const.tile([128, 4], F32, tag="red")
        ag = const.tile([128, 4 * NCORES], F32, tag="ag")
        me = const.tile([128, 4], F32, tag="me")    # mean0 mean1 msq0 msq1
        var = const.tile([128, 2], F32, tag="var")
        std = const.tile([128, 2], F32, tag="std")
        rstd = const.tile([128, 2], F32, tag="rstd")
        sc_b = const.tile([128, 4], F32, tag="scb")  # scale0 scale1 nbias0 nbias1

        for oc in range(2):
            nc.vector.tensor_reduce(red[:, oc:oc + 1], sums[oc][:],
                                    axis=mybir.AxisListType.X,
                                    op=mybir.AluOpType.add)
            nc.vector.tensor_reduce(red[:, 2 + oc:3 + oc], sqs[oc][:],
                                    axis=mybir.AxisListType.X,
                                    op=mybir.AluOpType.add)
        nc.sync.dma_start(st_in[:], red[:])
        if no_cc:
            nc.sync.dma_start(st_gather[0], st_in[:])
            for r in range(1, NCORES):
                nc.sync.dma_start(st_gather[r], st_in[:])
        else:
            nc.gpsimd.collective_compute(
                "AllGather", mybir.AluOpType.bypass,
                replica_groups=[list(range(NCORES))],
                ins=[st_in[:].opt()], outs=[st_gather[:].opt()])
        nc.sync.dma_start(
            ag[:], st_gather[:].rearrange("r p f -> p r f"))
        nc.vector.tensor_tensor(ag[:, 0:16], ag[:, 0:16], ag[:, 16:32],
                                mybir.AluOpType.add)
        nc.vector.tensor_tensor(ag[:, 0:8], ag[:, 0:8], ag[:, 8:16],
                                mybir.AluOpType.add)
        nc.vector.tensor_tensor(ag[:, 0:4], ag[:, 0:4], ag[:, 4:8],
                                mybir.AluOpType.add)

        nc.vector.tensor_scalar(me[:], ag[:, 0:4], 1.0 / NTOT, None,
                                mybir.AluOpType.mult)
        nc.vector.tensor_tensor(var[:], me[:, 0:2], me[:, 0:2],
                                mybir.AluOpType.mult)
        nc.vector.tensor_tensor(var[:], me[:, 2:4], var[:],
                                mybir.AluOpType.subtract)
        nc.vector.tensor_scalar(var[:], var[:], EPS, None,
                                mybir.AluOpType.add)
        nc.scalar.activation(std[:], var[:],
                             mybir.ActivationFunctionType.Sqrt)
        nc.vector.reciprocal(rstd[:], std[:])
        nc.vector.tensor_tensor(sc_b[:, 0:2], rstd[:], gb_sb[:, 0:2],
                                mybir.AluOpType.mult)
        nc.vector.tensor_tensor(sc_b[:, 2:4], me[:, 0:2], sc_b[:, 0:2],
                                mybir.AluOpType.mult)
        nc.vector.tensor_tensor(sc_b[:, 2:4], gb_sb[:, 2:4], sc_b[:, 2:4],
                                mybir.AluOpType.subtract)

        # ---- phase 2: affine + bf16 writeout (split DVE/ACT/Pool) ----
        for b in range(BPC):
            for oc in range(2):
                o_sb = ypool.tile([128, HW], BF16, tag="y", name=f"o{b}_{oc}")
                idx = 2 * b + oc
                if idx % 2 == 0:
                    nc.vector.tensor_scalar(
                        o_sb[:], res_tiles[b][oc][:],
                        sc_b[:, oc:oc + 1], sc_b[:, 2 + oc:3 + oc],
                        mybir.AluOpType.mult, mybir.AluOpType.add)
                elif idx % 4 == 1:
                    nc.scalar.activation(
                        o_sb[:], res_tiles[b][oc][:],
                        mybir.ActivationFunctionType.Identity,
                        bias=sc_b[:, 2 + oc:3 + oc],
                        scale=sc_b[:, oc:oc + 1])
                else:
                    nc.gpsimd.tensor_scalar(
                        o_sb[:], res_tiles[b][oc][:],
                        sc_b[:, oc:oc + 1], sc_b[:, 2 + oc:3 + oc],
                        mybir.AluOpType.mult, mybir.AluOpType.add)
                nc.sync.dma_start(out[b, 128 * oc:128 * (oc + 1), :], o_sb[:])

    nc.finalize()
    return nc


_NC_CACHE = []


def kernel(x, dw_w, pw_w, gamma, beta):
    import ml_dtypes
    x = np.ascontiguousarray(
        np.asarray(x, dtype=np.float32).astype(ml_dtypes.bfloat16)
    ).reshape(B, CIN, HW)
    dwk = np.ascontiguousarray(np.asarray(dw_w, dtype=np.float32)).reshape(9)
    pwT = np.ascontiguousarray(np.asarray(pw_w, dtype=np.float32).T)
    gamma = np.ascontiguousarray(np.asarray(gamma, dtype=np.float32))
    beta = np.ascontiguousarray(np.asarray(beta, dtype=np.float32))

    if not _NC_CACHE:
        _NC_CACHE.append(build_nc())
    nc = _NC_CACHE[0]

    in_maps = []
    for r in range(NCORES):
        shard = np.ascontiguousarray(x[r * BPC:(r + 1) * BPC])
        in_maps.append({"x": shard, "dwk": dwk, "pwT": pwT,
                        "gamma": gamma, "beta": beta})

    br = run_bass_kernel_spmd(nc, in_maps, list(range(NCORES)))
    outs = [np.asarray(br.results[r]["out"], dtype=np.float32)
            .reshape(BPC, COUT, H, W) for r in range(NCORES)]
    return np.concatenate(outs, axis=0)
